# revision 1
# baseline (speedup 1.0000x reference)
"""Segment softmax (GAT attention stage 4) on 8 TRN2 NeuronCores.

alpha_i = exp(e_i) / sum_{j: tgt_j == tgt_i} exp(e_j) — identical to the
reference: with e ~ N(0,1) the max-shift cancels exactly and the 1e-16
regularizer is negligible (every segment is non-empty w.o.p.).

Strategy: shard NODES across the 8 cores (each target node's edges live on
exactly one core), so there is no cross-core reduction at all. The host
ranks nodes by degree, deals them round-robin to cores (identical degree
profile per core), and packs each node's edges into a contiguous column
range of one SBUF partition row; 128 node positions form a "chunk", chunks
with similar capacity form uniform-[128, G, C] "slabs" (~2% padding).
Edge values ship as fp16 (25.6M-sample N(0,1) fits comfortably), alpha
returns as fp16; rel_l2 ~5e-4 vs the 2e-2 gate. Device work per slab,
tuned from measured per-op rates:

  - Segment sums: two dense 2x fold-adds (xt halves, tensor_tensor f16)
    shrink each chunk to C/4 columns, then one 1x grouped tensor_reduce.
    ~22us DVE vs ~27-33us for a full-width reduce.
  - Normalize: per-chunk scalar multiply split DVE/ACT by measured costs
    (DVE tensor_scalar ~0.25us, ACT Copy+scale ~0.63us); no GPSIMD — v4
    showed its tensor ops are ~7x slower and its SBUF-port contention
    inflates DVE by ~25%.
  - Pad-only lanes get slot0 = 0.0 from the host, so no eps pass.
  - All DMAs issue from the otherwise-idle SP (sync) sequencer.
"""

import numpy as np

P = 128
NCORES = 8
S_MAX = 6144  # max columns per slab (per partition)
G_MAX = 128  # max chunks per core
PAD = -60.0  # exp(PAD) == 0 in fp16

# measured per-chunk normalize costs, ns (fixed, per-column)
COST_DVE = (200.0, 0.27)
COST_ACT = (250.0, 1.45)

_CACHE = {}


def _plan(deg, num_nodes):
    """Node ranking, chunk capacities, slab grouping. Data-dependent."""
    N = num_nodes
    npc = -(-N // NCORES)  # node positions per core
    G = -(-npc // P)  # chunks per core
    order = np.argsort(-deg, kind="stable")
    deg_sorted = deg[order].astype(np.int64)
    caps = np.empty(G, dtype=np.int64)
    for g in range(G):
        lead = min(g * P * NCORES, N - 1)
        caps[g] = max(1, int(deg_sorted[lead]))
    smax = max(S_MAX, int(caps[0]))
    slabs = []  # (col_off, G_s, C_s)
    g = 0
    off = 0
    while g < G:
        C_s = -(-int(caps[g]) // 4) * 4  # %4 for the two fold levels
        G_s = 1
        while (
            g + G_s < G
            and (G_s + 1) * C_s <= smax
            and (C_s - caps[g + G_s]) <= max(4, C_s // 16)
        ):
            G_s += 1
        slabs.append((off, G_s, C_s))
        off += G_s * C_s
        g += G_s
    W = off
    chunk_off = np.empty(G, dtype=np.int64)
    g = 0
    for s_off, G_s, C_s in slabs:
        for k in range(G_s):
            chunk_off[g] = s_off + k * C_s
            g += 1
    return order, deg_sorted, slabs, W, chunk_off


def _mul_split(slabs):
    """Greedy engine assignment for the per-chunk normalize multiplies.

    Returns per-slab (n_act, n_gp): within a slab the first n_act chunks
    multiply on ACT, the next n_gp on GPSIMD, the rest on DVE. Initial
    loads seed each engine with its non-normalize work.
    """
    total_cols = sum(G_s * C_s for _, G_s, C_s in slabs)
    nslabs = len(slabs)
    # ns. ACT: slab exp at ~1.08 GHz effective. DVE: folds at 2x over
    # 3/4 of the columns, reduce at 1x over C/4, recip, per-op overhead.
    load = {
        "act": total_cols / 1.08 + nslabs * 250.0,
        "dve": total_cols * (0.75 * 0.52 + 0.25 * 1.04)
        + nslabs * (4 * 220.0 + 260.0),
    }
    cost = {"dve": COST_DVE, "act": COST_ACT}
    assign = []
    for _, G_s, C_s in slabs:
        n_act = 0
        for _ in range(G_s):
            best = None
            for eng in ("dve", "act"):
                f, v = cost[eng]
                t = load[eng] + f + v * C_s
                if best is None or t < best[1]:
                    best = (eng, t)
            eng = best[0]
            load[eng] = best[1]
            if eng == "act":
                n_act += 1
        assign.append((n_act, 0))
    return assign


def _build(slabs, W):
    import concourse.mybir as mybir
    from concourse import bacc
    from concourse.tile import TileContext

    nc = bacc.Bacc(None, target_bir_lowering=False)
    ev = nc.dram_tensor("ev", [P, W], mybir.dt.float16, kind="ExternalInput")
    av = nc.dram_tensor("av", [P, W], mybir.dt.float16, kind="ExternalOutput")

    smax = max(S_MAX, max(C for _, _, C in slabs))
    split = _mul_split(slabs)
    with TileContext(nc) as tc:
        with tc.tile_pool(name="sbuf", bufs=4) as pool:
            for (off, G_s, C_s), (n_act, n_gp) in zip(slabs, split):
                S = G_s * C_s
                et = pool.tile([P, smax], mybir.dt.float16, tag="e")
                nc.sync.dma_start(out=et[:, :S], in_=ev[:, off : off + S])
                xt = pool.tile([P, smax], mybir.dt.float16, tag="x")
                nc.scalar.activation(
                    xt[:, :S], et[:, :S], mybir.ActivationFunctionType.Exp
                )
                st = pool.tile([P, G_MAX], mybir.dt.float32, tag="s")
                if C_s >= 8:
                    h = C_s // 2
                    hq = C_s // 4
                    x3 = xt[:, :S].rearrange("p (g c) -> p g c", g=G_s)
                    yt = pool.tile([P, smax // 2], mybir.dt.float16, tag="y")
                    y3 = yt[:, : G_s * h].rearrange("p (g c) -> p g c", g=G_s)
                    nc.vector.tensor_add(out=y3, in0=x3[:, :, :h], in1=x3[:, :, h:])
                    zt = pool.tile([P, smax // 4], mybir.dt.float16, tag="z")
                    z3 = zt[:, : G_s * hq].rearrange("p (g c) -> p g c", g=G_s)
                    nc.vector.tensor_add(out=z3, in0=y3[:, :, :hq], in1=y3[:, :, hq:])
                    nc.vector.tensor_reduce(
                        out=st[:, :G_s],
                        in_=z3,
                        axis=mybir.AxisListType.X,
                        op=mybir.AluOpType.add,
                    )
                else:
                    nc.vector.tensor_reduce(
                        out=st[:, :G_s],
                        in_=xt[:, :S].rearrange("p (g c) -> p g c", g=G_s),
                        axis=mybir.AxisListType.X,
                        op=mybir.AluOpType.add,
                    )
                qt = pool.tile([P, G_MAX], mybir.dt.float32, tag="q")
                nc.vector.reciprocal(out=qt[:, :G_s], in_=st[:, :G_s])
                at = pool.tile([P, smax], mybir.dt.float16, tag="a")
                for g in range(G_s):
                    o = slice(g * C_s, (g + 1) * C_s)
                    if g < n_act:
                        nc.scalar.mul(at[:, o], xt[:, o], qt[:, g : g + 1])
                    else:
                        nc.vector.tensor_scalar_mul(
                            out=at[:, o], in0=xt[:, o], scalar1=qt[:, g : g + 1]
                        )
                nc.sync.dma_start(out=av[:, off : off + S], in_=at[:, :S])
    nc.compile()
    return nc


def _prepare(e, tgt, num_nodes):
    """Host-side pack: (per-core ev arrays, scatter metadata for unpack)."""
    E = e.shape[0]
    N = num_nodes
    deg = np.bincount(tgt, minlength=N).astype(np.int64)
    order, deg_sorted, slabs, W, chunk_off = _plan(deg, N)

    rankpos = np.empty(N, dtype=np.int64)
    rankpos[order] = np.arange(N, dtype=np.int64)
    r = rankpos[tgt]  # [E] degree-rank of each edge's target
    sidx = np.argsort(r, kind="stable")  # edges grouped by rank
    rs = r[sidx]
    starts = np.concatenate(([0], np.cumsum(deg_sorted[:-1])))
    j = np.arange(E, dtype=np.int64) - starts[rs]  # slot within node
    core = rs % NCORES
    pos = rs // NCORES
    gidx = pos // P
    lane = pos % P
    col = chunk_off[gidx] + j
    flat = lane * W + col

    ev = np.full((NCORES, P * W), PAD, dtype=np.float16)
    # lanes with no edges (pad node positions and zero-degree nodes) get
    # slot0 = 0.0 so their segment sum is exp(0) = 1, keeping the
    # reciprocal finite without an eps pass on device.
    npc = -(-N // NCORES)
    G = -(-npc // P)
    allpos = np.arange(NCORES * G * P, dtype=np.int64)
    acore = allpos % NCORES
    apos = allpos // NCORES
    adeg = np.zeros(NCORES * G * P, dtype=np.int64)
    ranked = np.arange(min(N, NCORES * G * P), dtype=np.int64)
    adeg[ranked] = deg_sorted[: len(ranked)]
    empty = adeg == 0
    ecore = acore[empty]
    eg = apos[empty] // P
    elane = apos[empty] % P
    ev[ecore, elane * W + chunk_off[eg]] = 0.0
    ev[core, flat] = e[sidx].astype(np.float16)
    return ev, slabs, W, sidx, core, flat


def kernel(e, edge_index, num_nodes):
    from concourse.bass_utils import run_bass_kernel_spmd

    e = np.ascontiguousarray(np.asarray(e, dtype=np.float32))
    tgt = np.asarray(edge_index)[1].astype(np.int64)
    N = int(num_nodes)
    E = e.shape[0]

    ev, slabs, W, sidx, core, flat = _prepare(e, tgt, N)

    key = (tuple(slabs), W)
    if key not in _CACHE:
        _CACHE[key] = _build(slabs, W)
    nc = _CACHE[key]

    in_maps = [{"ev": ev[c].reshape(P, W)} for c in range(NCORES)]
    res = run_bass_kernel_spmd(nc, in_maps, core_ids=list(range(NCORES)))

    av = np.stack([res.results[c]["av"].reshape(-1) for c in range(NCORES)])
    alpha = np.empty(E, dtype=np.float32)
    alpha[sidx] = av[core, flat].astype(np.float32)
    return alpha



# revision 2
# speedup vs baseline: 1.1337x; 1.1337x over previous
"""Segment softmax (GAT attention stage 4) on 8 TRN2 NeuronCores.

alpha_i = exp(e_i) / sum_{j: tgt_j == tgt_i} exp(e_j) — identical to the
reference: with e ~ N(0,1) the max-shift cancels exactly and the 1e-16
regularizer is negligible (every segment is non-empty w.o.p.).

Strategy: shard NODES across the 8 cores (each target node's edges live on
exactly one core), so there is no cross-core reduction at all. The host
ranks nodes by degree, deals them round-robin to cores (identical degree
profile per core), and packs each node's edges into a contiguous column
range of one SBUF partition row; 128 node positions form a "chunk", chunks
with similar capacity form uniform-[128, G, C] "slabs" (~3% padding).

v2 changes vs the 60-70us baseline (trace-driven):
  - Edge logits ship as symmetric int8 (q = max|e|/127); ACT fuses the
    dequant into the table op: x = Exp(q * u). The missing zero-offset
    cancels in the softmax. Halves input HBM traffic; measured end-to-end
    rel_l2 ~1.2e-2 vs the 2e-2 gate on the seeded data. Pad byte -128
    gives exp ~ 4e-3, adding < 3e-4 relative to any segment sum, and
    keeps empty-lane sums positive (no eps pass needed).
  - The ~98 per-chunk normalize multiplies (24us DVE + 24us ACT in the
    v1 trace) are replaced by ONE slab-wide DVE tensor_tensor with a
    stride-0 (broadcast) AP on the reciprocal operand, plus a few
    per-chunk ACT Copy+scale ops for engine balance.
  - Three fold-add levels (C%8) ahead of the grouped tensor_reduce.
  - Output alpha returns as fp16.
"""

import numpy as np

P = 128
NCORES = 8
S_MAX = 6144  # max columns per slab (per partition)
G_MAX = 192  # max chunks per core

# measured per-op costs, ns (fixed, per-column)
ACT_EXP = (250.0, 0.93)
ACT_MUL = (250.0, 1.45)
DVE_TT = (200.0, 0.57)  # tensor_tensor fp16 per output column
DVE_RED = (200.0, 1.45)  # grouped tensor_reduce per input column
DVE_FIX = 200.0

_CACHE = {}


def _plan(deg, num_nodes):
    """Node ranking, chunk capacities, slab grouping. Data-dependent."""
    N = num_nodes
    npc = -(-N // NCORES)  # node positions per core
    G = -(-npc // P)  # chunks per core
    order = np.argsort(-deg, kind="stable")
    deg_sorted = deg[order].astype(np.int64)
    caps = np.empty(G, dtype=np.int64)
    for g in range(G):
        lead = min(g * P * NCORES, N - 1)
        caps[g] = max(1, int(deg_sorted[lead]))
    smax = max(S_MAX, int(-(-int(caps[0]) // 8) * 8))
    slabs = []  # (col_off, G_s, C_s)
    g = 0
    off = 0
    while g < G:
        C_s = -(-int(caps[g]) // 8) * 8  # %8 for the three fold levels
        G_s = 1
        while (
            g + G_s < G
            and (G_s + 1) * C_s <= smax
            and (C_s - caps[g + G_s]) <= max(8, C_s // 16)
        ):
            G_s += 1
        slabs.append((off, G_s, C_s))
        off += G_s * C_s
        g += G_s
    W = off
    chunk_off = np.empty(G, dtype=np.int64)
    g = 0
    for s_off, G_s, C_s in slabs:
        for k in range(G_s):
            chunk_off[g] = s_off + k * C_s
            g += 1
    return order, deg_sorted, slabs, W, chunk_off


def _mul_split(slabs):
    """Per-slab n_act: the first n_act chunks normalize on ACT (per-chunk
    Copy+scale), the rest in one broadcast DVE tensor_tensor. Greedy
    balance of predicted engine loads."""
    total_cols = sum(G_s * C_s for _, G_s, C_s in slabs)
    ns = len(slabs)
    act = ns * ACT_EXP[0] + total_cols * ACT_EXP[1]
    # folds (3 levels) + grouped reduce + recip/copy + full-width bmul
    dve = (
        total_cols * (0.5 + 0.25 + 0.125) * DVE_TT[1]
        + total_cols * 0.125 * DVE_RED[1]
        + ns * (3 * DVE_TT[0] + DVE_RED[0] + 3 * DVE_FIX)
        + total_cols * DVE_TT[1]
    )
    n_act = [0] * ns
    # move chunks (largest C first) from the DVE bmul to ACT while the
    # DVE load exceeds the ACT load by more than the swap cost
    idx = sorted(range(ns), key=lambda s: -slabs[s][2])
    moved = True
    while moved:
        moved = False
        for s in idx:
            _, G_s, C_s = slabs[s]
            if n_act[s] >= G_s - 1:
                continue
            d_dve = C_s * DVE_TT[1]
            d_act = ACT_MUL[0] + C_s * ACT_MUL[1]
            if dve - act > d_dve + d_act:
                n_act[s] += 1
                dve -= d_dve
                act += d_act
                moved = True
    return n_act


def _build(slabs, W, q):
    import concourse.mybir as mybir
    from concourse import bacc
    from concourse.tile import TileContext

    nc = bacc.Bacc(None, target_bir_lowering=False)
    ev = nc.dram_tensor("ev", [P, W], mybir.dt.int8, kind="ExternalInput")
    av = nc.dram_tensor("av", [P, W], mybir.dt.float16, kind="ExternalOutput")

    smax = max(S_MAX, max(C for _, _, C in slabs))
    n_acts = _mul_split(slabs)
    with TileContext(nc) as tc:
        with tc.tile_pool(name="sbuf", bufs=4) as pool:
            for (off, G_s, C_s), n_act in zip(slabs, n_acts):
                S = G_s * C_s
                et = pool.tile([P, smax], mybir.dt.int8, tag="e")
                nc.sync.dma_start(out=et[:, :S], in_=ev[:, off : off + S])
                xt = pool.tile([P, smax], mybir.dt.float16, tag="x")
                nc.scalar.activation(
                    xt[:, :S],
                    et[:, :S],
                    mybir.ActivationFunctionType.Exp,
                    scale=float(q),
                )
                st = pool.tile([P, G_MAX], mybir.dt.float32, tag="s")
                x3 = xt[:, :S].rearrange("p (g c) -> p g c", g=G_s)
                if C_s % 8 == 0 and C_s >= 32:
                    h, hq, he = C_s // 2, C_s // 4, C_s // 8
                    yt = pool.tile([P, smax // 2], mybir.dt.float16, tag="y")
                    y3 = yt[:, : G_s * h].rearrange("p (g c) -> p g c", g=G_s)
                    nc.vector.tensor_add(out=y3, in0=x3[:, :, :h], in1=x3[:, :, h:])
                    zt = pool.tile([P, smax // 4], mybir.dt.float16, tag="z")
                    z3 = zt[:, : G_s * hq].rearrange("p (g c) -> p g c", g=G_s)
                    nc.vector.tensor_add(out=z3, in0=y3[:, :, :hq], in1=y3[:, :, hq:])
                    wt = pool.tile([P, smax // 8], mybir.dt.float16, tag="w")
                    w3 = wt[:, : G_s * he].rearrange("p (g c) -> p g c", g=G_s)
                    nc.vector.tensor_add(out=w3, in0=z3[:, :, :he], in1=z3[:, :, he:])
                    red_in = w3
                elif C_s % 4 == 0 and C_s >= 8:
                    h, hq = C_s // 2, C_s // 4
                    yt = pool.tile([P, smax // 2], mybir.dt.float16, tag="y")
                    y3 = yt[:, : G_s * h].rearrange("p (g c) -> p g c", g=G_s)
                    nc.vector.tensor_add(out=y3, in0=x3[:, :, :h], in1=x3[:, :, h:])
                    zt = pool.tile([P, smax // 4], mybir.dt.float16, tag="z")
                    z3 = zt[:, : G_s * hq].rearrange("p (g c) -> p g c", g=G_s)
                    nc.vector.tensor_add(out=z3, in0=y3[:, :, :hq], in1=y3[:, :, hq:])
                    red_in = z3
                else:
                    red_in = x3
                nc.vector.tensor_reduce(
                    out=st[:, :G_s],
                    in_=red_in,
                    axis=mybir.AxisListType.X,
                    op=mybir.AluOpType.add,
                )
                qt32 = pool.tile([P, G_MAX], mybir.dt.float32, tag="q32")
                nc.vector.reciprocal(out=qt32[:, :G_s], in_=st[:, :G_s])
                qt = pool.tile([P, G_MAX], mybir.dt.float16, tag="q16")
                with nc.allow_low_precision(reason="1/s fits fp16; gate 2e-2"):
                    nc.vector.tensor_copy(out=qt[:, :G_s], in_=qt32[:, :G_s])
                at = pool.tile([P, smax], mybir.dt.float16, tag="a")
                for g in range(n_act):
                    o = slice(g * C_s, (g + 1) * C_s)
                    nc.scalar.mul(at[:, o], xt[:, o], qt32[:, g : g + 1])
                if n_act < G_s:
                    Gr = G_s - n_act
                    o = slice(n_act * C_s, S)
                    nc.vector.tensor_mul(
                        out=at[:, o].rearrange("p (g c) -> p g c", g=Gr),
                        in0=xt[:, o].rearrange("p (g c) -> p g c", g=Gr),
                        in1=qt[:, n_act:G_s].to_broadcast([P, Gr, C_s]),
                    )
                nc.sync.dma_start(out=av[:, off : off + S], in_=at[:, :S])
    nc.compile()
    return nc


def _prepare(e, tgt, num_nodes):
    """Host-side pack: (per-core int8 arrays, scale, scatter metadata)."""
    E = e.shape[0]
    N = num_nodes
    deg = np.bincount(tgt, minlength=N).astype(np.int64)
    order, deg_sorted, slabs, W, chunk_off = _plan(deg, N)

    q = float(np.abs(e).max()) / 127.0
    e8 = np.clip(np.rint(e * (1.0 / q)), -127, 127).astype(np.int8)

    rankpos = np.empty(N, dtype=np.int64)
    rankpos[order] = np.arange(N, dtype=np.int64)
    r = rankpos[tgt]  # [E] degree-rank of each edge's target
    sidx = np.argsort(r, kind="stable")  # edges grouped by rank
    rs = r[sidx]
    starts = np.concatenate(([0], np.cumsum(deg_sorted[:-1])))
    j = np.arange(E, dtype=np.int64) - starts[rs]  # slot within node
    core = rs % NCORES
    pos = rs // NCORES
    gidx = pos // P
    lane = pos % P
    col = chunk_off[gidx] + j
    flat = lane * W + col

    ev = np.full((NCORES, P * W), -128, dtype=np.int8)
    ev[core, flat] = e8[sidx]
    return ev, slabs, W, q, sidx, core, flat


def kernel(e, edge_index, num_nodes):
    from concourse.bass_utils import run_bass_kernel_spmd

    e = np.ascontiguousarray(np.asarray(e, dtype=np.float32))
    tgt = np.asarray(edge_index)[1].astype(np.int64)
    N = int(num_nodes)
    E = e.shape[0]

    ev, slabs, W, q, sidx, core, flat = _prepare(e, tgt, N)

    key = (tuple(slabs), W, round(q, 9))
    if key not in _CACHE:
        _CACHE[key] = _build(slabs, W, q)
    nc = _CACHE[key]

    in_maps = [{"ev": ev[c].reshape(P, W)} for c in range(NCORES)]
    res = run_bass_kernel_spmd(nc, in_maps, core_ids=list(range(NCORES)))

    av = np.stack([res.results[c]["av"].reshape(-1) for c in range(NCORES)])
    alpha = np.empty(E, dtype=np.float32)
    alpha[sidx] = av[core, flat].astype(np.float32)
    return alpha


# revision 6
# speedup vs baseline: 1.2792x; 1.1283x over previous
"""Segment softmax (GAT attention stage 4) on 8 TRN2 NeuronCores.

alpha_i = exp(e_i) / sum_{j: tgt_j == tgt_i} exp(e_j) — identical to the
reference: with e ~ N(0,1) the max-shift cancels exactly and the 1e-16
regularizer is negligible (every segment is non-empty w.o.p.).

Strategy: shard NODES across the 8 cores (each target node's edges live on
exactly one core), so there is no cross-core reduction at all. The host
ranks nodes by degree, deals them round-robin to cores (identical degree
profile per core), and packs each node's edges into a contiguous column
range of one SBUF partition row; 128 node positions form a "chunk", chunks
with similar capacity form uniform-[128, G, C] "slabs" (~3% padding).

v2 changes vs the 60-70us baseline (trace-driven):
  - Edge logits ship as symmetric int8 (q = max|e|/127); ACT fuses the
    dequant into the table op: x = Exp(q * u). The missing zero-offset
    cancels in the softmax. Halves input HBM traffic; measured end-to-end
    rel_l2 ~1.2e-2 vs the 2e-2 gate on the seeded data. Pad byte -128
    gives exp ~ 4e-3, adding < 3e-4 relative to any segment sum, and
    keeps empty-lane sums positive (no eps pass needed).
  - The ~98 per-chunk normalize multiplies (24us DVE + 24us ACT in the
    v1 trace) are replaced by ONE slab-wide DVE tensor_tensor with a
    stride-0 (broadcast) AP on the reciprocal operand, plus a few
    per-chunk ACT Copy+scale ops for engine balance.
  - Three fold-add levels (C%8) ahead of the grouped tensor_reduce.
  - Output alpha returns as fp16.
"""

import numpy as np

P = 128
NCORES = 8
S_MAX = 6144  # max columns per slab (per partition)
G_MAX = 192  # max chunks per core

# measured per-op costs, ns (fixed, per-column)
ACT_EXP = (350.0, 0.88)
ACT_MUL = (600.0, 0.0)  # ~flat for C in [190, 350]
DVE_TT = (140.0, 0.52)  # tensor_tensor fp16 packed (2x) per output column
DVE_BM = (150.0, 0.547)  # pair-broadcast tensor_tensor (2x)
DVE_RED = (170.0, 1.24)  # grouped tensor_reduce per input column
DVE_FIX = 200.0

_CACHE = {}


def _plan(deg, num_nodes):
    """Node ranking, chunk capacities, slab grouping. Data-dependent."""
    N = num_nodes
    npc = -(-N // NCORES)  # node positions per core
    G = -(-npc // P)  # chunks per core
    order = np.argsort(-deg, kind="stable")
    deg_sorted = deg[order].astype(np.int64)
    caps = np.empty(G, dtype=np.int64)
    for g in range(G):
        lead = min(g * P * NCORES, N - 1)
        caps[g] = max(1, int(deg_sorted[lead]))
    smax = max(S_MAX, int(-(-int(caps[0]) // 8) * 8))
    slabs = []  # (col_off, G_s, C_s)
    g = 0
    off = 0
    while g < G:
        C_s = -(-int(caps[g]) // 8) * 8  # %8 for the three fold levels
        G_s = 1
        while (
            g + G_s < G
            and (G_s + 1) * C_s <= smax
            and (C_s - caps[g + G_s]) <= max(8, C_s // 16)
        ):
            G_s += 1
        slabs.append((off, G_s, C_s))
        off += G_s * C_s
        g += G_s
    W = off
    chunk_off = np.empty(G, dtype=np.int64)
    g = 0
    for s_off, G_s, C_s in slabs:
        for k in range(G_s):
            chunk_off[g] = s_off + k * C_s
            g += 1
    return order, deg_sorted, slabs, W, chunk_off


def _order(slabs):
    """Device processing order: smallest slab first (fast pipeline ramp),
    then descending size, ending on the second-smallest (short tail)."""
    idx = sorted(range(len(slabs)), key=lambda s: slabs[s][1] * slabs[s][2])
    if len(idx) <= 2:
        return idx
    return [idx[0]] + idx[:0:-1]


def _mul_split(slabs, proc):
    """Per-slab n_act: the first n_act chunks normalize on ACT (per-chunk
    Copy+scale), the rest in one pair-broadcast DVE tensor_tensor. Greedy
    balance of predicted engine loads; the last two processed slabs stay
    fully on DVE so the ACT queue drains early (short tail)."""
    total_cols = sum(G_s * C_s for _, G_s, C_s in slabs)
    ns = len(slabs)
    act = ns * ACT_EXP[0] + total_cols * ACT_EXP[1]
    # folds (3 levels) + grouped reduce + recip/dup-cast + 2x bmul
    dve = (
        total_cols * (0.5 + 0.25 + 0.125) * DVE_TT[1]
        + total_cols * 0.125 * DVE_RED[1]
        + ns * (3 * DVE_TT[0] + DVE_RED[0] + 2 * DVE_FIX + DVE_BM[0])
        + total_cols * DVE_BM[1]
    )
    n_act = [0] * ns
    late = set(proc[-2:]) if len(proc) > 3 else set()
    idx = [s for s in sorted(range(ns), key=lambda s: -slabs[s][2]) if s not in late]
    moved = True
    while moved and idx:
        moved = False
        for s in idx:
            _, G_s, C_s = slabs[s]
            if n_act[s] >= G_s - 1:
                continue
            d_dve = C_s * DVE_BM[1]
            d_act = ACT_MUL[0] + C_s * ACT_MUL[1]
            if dve - act > d_dve + d_act:
                n_act[s] += 1
                dve -= d_dve
                act += d_act
                moved = True
    return n_act


def _build(slabs, W, q):
    import concourse.mybir as mybir
    from concourse import bacc
    from concourse.tile import TileContext

    nc = bacc.Bacc(None, target_bir_lowering=False)
    ev = nc.dram_tensor("ev", [P, W], mybir.dt.int8, kind="ExternalInput")
    av = nc.dram_tensor("av", [P, W], mybir.dt.float16, kind="ExternalOutput")

    smax = max(S_MAX, max(C for _, _, C in slabs))
    proc = _order(slabs)
    n_acts = _mul_split(slabs, proc)
    with TileContext(nc) as tc:
        with tc.tile_pool(name="sbuf", bufs=4) as pool:
            for si in proc:
                (off, G_s, C_s), n_act = slabs[si], n_acts[si]
                S = G_s * C_s
                et = pool.tile([P, smax], mybir.dt.int8, tag="e")
                nc.sync.dma_start(out=et[:, :S], in_=ev[:, off : off + S])
                xt = pool.tile([P, smax], mybir.dt.float16, tag="x")
                nc.scalar.activation(
                    xt[:, :S],
                    et[:, :S],
                    mybir.ActivationFunctionType.Exp,
                    scale=float(q),
                )
                st = pool.tile([P, G_MAX], mybir.dt.float32, tag="s")
                x3 = xt[:, :S].rearrange("p (g c) -> p g c", g=G_s)
                if C_s % 8 == 0 and C_s >= 32:
                    h, hq, he = C_s // 2, C_s // 4, C_s // 8
                    yt = pool.tile([P, smax // 2], mybir.dt.float16, tag="y")
                    y3 = yt[:, : G_s * h].rearrange("p (g c) -> p g c", g=G_s)
                    nc.vector.tensor_add(out=y3, in0=x3[:, :, :h], in1=x3[:, :, h:])
                    zt = pool.tile([P, smax // 4], mybir.dt.float16, tag="z")
                    z3 = zt[:, : G_s * hq].rearrange("p (g c) -> p g c", g=G_s)
                    nc.vector.tensor_add(out=z3, in0=y3[:, :, :hq], in1=y3[:, :, hq:])
                    wt = pool.tile([P, smax // 8], mybir.dt.float16, tag="w")
                    w3 = wt[:, : G_s * he].rearrange("p (g c) -> p g c", g=G_s)
                    nc.vector.tensor_add(out=w3, in0=z3[:, :, :he], in1=z3[:, :, he:])
                    red_in = w3
                elif C_s % 4 == 0 and C_s >= 8:
                    h, hq = C_s // 2, C_s // 4
                    yt = pool.tile([P, smax // 2], mybir.dt.float16, tag="y")
                    y3 = yt[:, : G_s * h].rearrange("p (g c) -> p g c", g=G_s)
                    nc.vector.tensor_add(out=y3, in0=x3[:, :, :h], in1=x3[:, :, h:])
                    zt = pool.tile([P, smax // 4], mybir.dt.float16, tag="z")
                    z3 = zt[:, : G_s * hq].rearrange("p (g c) -> p g c", g=G_s)
                    nc.vector.tensor_add(out=z3, in0=y3[:, :, :hq], in1=y3[:, :, hq:])
                    red_in = z3
                else:
                    red_in = x3
                nc.vector.tensor_reduce(
                    out=st[:, :G_s],
                    in_=red_in,
                    axis=mybir.AxisListType.X,
                    op=mybir.AluOpType.add,
                )
                qt32 = pool.tile([P, G_MAX], mybir.dt.float32, tag="q32")
                nc.vector.reciprocal(out=qt32[:, :G_s], in_=st[:, :G_s])
                # duplicated-pair fp16 reciprocals: the bmul broadcast AP
                # gets a packed (stride-1, count-2) last dim, which keeps
                # the DVE in 2x mode (a plain stride-0 operand drops to 1x)
                qt2 = pool.tile([P, 2 * G_MAX], mybir.dt.float16, tag="q2")
                with nc.allow_low_precision(reason="1/s fits fp16; gate 2e-2"):
                    nc.vector.tensor_copy(
                        out=qt2[:, : 2 * G_s].rearrange("p (g i) -> p g i", g=G_s),
                        in_=qt32[:, :G_s]
                        .rearrange("p g -> p g ()")
                        .to_broadcast([P, G_s, 2]),
                    )
                at = pool.tile([P, smax], mybir.dt.float16, tag="a")
                for g in range(n_act):
                    o = slice(g * C_s, (g + 1) * C_s)
                    nc.scalar.mul(at[:, o], xt[:, o], qt32[:, g : g + 1])
                if n_act < G_s:
                    Gr = G_s - n_act
                    o = slice(n_act * C_s, S)
                    nc.vector.tensor_mul(
                        out=at[:, o].rearrange(
                            "p (g c2 i) -> p g c2 i", g=Gr, i=2
                        ),
                        in0=xt[:, o].rearrange(
                            "p (g c2 i) -> p g c2 i", g=Gr, i=2
                        ),
                        in1=qt2[:, 2 * n_act : 2 * G_s]
                        .rearrange("p (g i) -> p g () i", g=Gr)
                        .to_broadcast([P, Gr, C_s // 2, 2]),
                    )
                nc.sync.dma_start(out=av[:, off : off + S], in_=at[:, :S])
    nc.compile()
    return nc


def _prepare(e, tgt, num_nodes):
    """Host-side pack: (per-core int8 arrays, scale, scatter metadata)."""
    E = e.shape[0]
    N = num_nodes
    deg = np.bincount(tgt, minlength=N).astype(np.int64)
    order, deg_sorted, slabs, W, chunk_off = _plan(deg, N)

    q = float(np.abs(e).max()) / 127.0
    e8 = np.clip(np.rint(e * (1.0 / q)), -127, 127).astype(np.int8)

    rankpos = np.empty(N, dtype=np.int64)
    rankpos[order] = np.arange(N, dtype=np.int64)
    r = rankpos[tgt]  # [E] degree-rank of each edge's target
    sidx = np.argsort(r, kind="stable")  # edges grouped by rank
    rs = r[sidx]
    starts = np.concatenate(([0], np.cumsum(deg_sorted[:-1])))
    j = np.arange(E, dtype=np.int64) - starts[rs]  # slot within node
    core = rs % NCORES
    pos = rs // NCORES
    gidx = pos // P
    lane = pos % P
    col = chunk_off[gidx] + j
    flat = lane * W + col

    ev = np.full((NCORES, P * W), -128, dtype=np.int8)
    ev[core, flat] = e8[sidx]
    return ev, slabs, W, q, sidx, core, flat


def kernel(e, edge_index, num_nodes):
    from concourse.bass_utils import run_bass_kernel_spmd

    e = np.ascontiguousarray(np.asarray(e, dtype=np.float32))
    tgt = np.asarray(edge_index)[1].astype(np.int64)
    N = int(num_nodes)
    E = e.shape[0]

    ev, slabs, W, q, sidx, core, flat = _prepare(e, tgt, N)

    key = (tuple(slabs), W, round(q, 9))
    if key not in _CACHE:
        _CACHE[key] = _build(slabs, W, q)
    nc = _CACHE[key]

    in_maps = [{"ev": ev[c].reshape(P, W)} for c in range(NCORES)]
    res = run_bass_kernel_spmd(nc, in_maps, core_ids=list(range(NCORES)))

    av = np.stack([res.results[c]["av"].reshape(-1) for c in range(NCORES)])
    alpha = np.empty(E, dtype=np.float32)
    alpha[sidx] = av[core, flat].astype(np.float32)
    return alpha


# revision 8
# speedup vs baseline: 1.3046x; 1.0199x over previous
"""Segment softmax (GAT attention stage 4) on 8 TRN2 NeuronCores.

alpha_i = exp(e_i) / sum_{j: tgt_j == tgt_i} exp(e_j) — identical to the
reference: with e ~ N(0,1) the max-shift cancels exactly and the 1e-16
regularizer is negligible (every segment is non-empty w.o.p.).

Strategy: shard NODES across the 8 cores (each target node's edges live on
exactly one core), so there is no cross-core reduction at all. The host
ranks nodes by degree, deals them round-robin to cores (identical degree
profile per core), and packs each node's edges into a contiguous column
range of one SBUF partition row; 128 node positions form a "chunk", chunks
with similar capacity form uniform-[128, G, C] "slabs" (~3% padding).

v2 changes vs the 60-70us baseline (trace-driven):
  - Edge logits ship as symmetric int8 (q = max|e|/127); ACT fuses the
    dequant into the table op: x = Exp(q * u). The missing zero-offset
    cancels in the softmax. Halves input HBM traffic; measured end-to-end
    rel_l2 ~1.2e-2 vs the 2e-2 gate on the seeded data. Pad byte -128
    gives exp ~ 4e-3, adding < 3e-4 relative to any segment sum, and
    keeps empty-lane sums positive (no eps pass needed).
  - The ~98 per-chunk normalize multiplies (24us DVE + 24us ACT in the
    v1 trace) are replaced by ONE slab-wide DVE tensor_tensor with a
    stride-0 (broadcast) AP on the reciprocal operand, plus a few
    per-chunk ACT Copy+scale ops for engine balance.
  - Three fold-add levels (C%8) ahead of the grouped tensor_reduce.
  - Output alpha returns as fp16.
"""

import numpy as np

P = 128
NCORES = 8
S_MAX = 6144  # max columns per slab (per partition)
G_MAX = 192  # max chunks per core

# measured per-op costs, ns (fixed, per-column)
ACT_EXP = (350.0, 0.88)
ACT_MUL = (600.0, 0.0)  # ~flat for C in [190, 350]
DVE_TT = (140.0, 0.52)  # tensor_tensor fp16 packed (2x) per output column
DVE_BM = (150.0, 0.547)  # pair-broadcast tensor_tensor (2x)
DVE_RED = (170.0, 1.24)  # grouped tensor_reduce per input column
DVE_FIX = 200.0

_CACHE = {}


def _plan(deg, num_nodes):
    """Node ranking, chunk capacities, slab grouping. Data-dependent."""
    N = num_nodes
    npc = -(-N // NCORES)  # node positions per core
    G = -(-npc // P)  # chunks per core
    order = np.argsort(-deg, kind="stable")
    deg_sorted = deg[order].astype(np.int64)
    caps = np.empty(G, dtype=np.int64)
    for g in range(G):
        lead = min(g * P * NCORES, N - 1)
        caps[g] = max(1, int(deg_sorted[lead]))
    smax = max(S_MAX, int(-(-int(caps[0]) // 8) * 8))
    slabs = []  # (col_off, G_s, C_s)
    g = 0
    off = 0
    while g < G:
        C_s = -(-int(caps[g]) // 8) * 8  # %8 for the three fold levels
        G_s = 1
        while (
            g + G_s < G
            and (G_s + 1) * C_s <= smax
            and (C_s - caps[g + G_s]) <= max(8, C_s // 16)
        ):
            G_s += 1
        slabs.append((off, G_s, C_s))
        off += G_s * C_s
        g += G_s
    W = off
    chunk_off = np.empty(G, dtype=np.int64)
    g = 0
    for s_off, G_s, C_s in slabs:
        for k in range(G_s):
            chunk_off[g] = s_off + k * C_s
            g += 1
    return order, deg_sorted, slabs, W, chunk_off


def _order(slabs):
    """Device processing order: smallest slab first (fast pipeline ramp),
    then descending size, ending on the second-smallest (short tail)."""
    idx = sorted(range(len(slabs)), key=lambda s: slabs[s][1] * slabs[s][2])
    if len(idx) <= 2:
        return idx
    return [idx[0]] + idx[:0:-1]


def _mul_split(slabs, proc):
    """Per-slab n_act: the first n_act chunks normalize on ACT (per-chunk
    Copy+scale), the rest in one pair-broadcast DVE tensor_tensor. Greedy
    balance of predicted engine loads; the last two processed slabs stay
    fully on DVE so the ACT queue drains early (short tail)."""
    total_cols = sum(G_s * C_s for _, G_s, C_s in slabs)
    ns = len(slabs)
    act = ns * ACT_EXP[0] + total_cols * ACT_EXP[1]
    # folds (3 levels) + grouped reduce + recip/dup-cast + 2x bmul
    dve = (
        total_cols * (0.5 + 0.25 + 0.125) * DVE_TT[1]
        + total_cols * 0.125 * DVE_RED[1]
        + ns * (3 * DVE_TT[0] + DVE_RED[0] + 2 * DVE_FIX + DVE_BM[0])
        + total_cols * DVE_BM[1]
    )
    n_act = [0] * ns
    late = set(proc[-2:]) if len(proc) > 3 else set()
    idx = [s for s in sorted(range(ns), key=lambda s: -slabs[s][2]) if s not in late]
    moved = True
    while moved and idx:
        moved = False
        for s in idx:
            _, G_s, C_s = slabs[s]
            if n_act[s] >= G_s - 1:
                continue
            d_dve = C_s * DVE_BM[1]
            d_act = ACT_MUL[0] + C_s * ACT_MUL[1]
            if dve - act > d_dve + d_act:
                n_act[s] += 1
                dve -= d_dve
                act += d_act
                moved = True
    return n_act


def _build(slabs, W, q):
    import concourse.mybir as mybir
    from concourse import bacc
    from concourse.tile import TileContext

    nc = bacc.Bacc(None, target_bir_lowering=False)
    ev = nc.dram_tensor("ev", [P, W], mybir.dt.int8, kind="ExternalInput")
    av = nc.dram_tensor("av", [P, W], mybir.dt.float16, kind="ExternalOutput")

    smax = max(S_MAX, max(C for _, _, C in slabs))
    proc = _order(slabs)
    n_acts = _mul_split(slabs, proc)
    with TileContext(nc) as tc:
        with tc.tile_pool(name="sbuf", bufs=4) as pool:

            def load_exp(si):
                """Stage 1: input DMA + slab-wide dequantizing exp."""
                off, G_s, C_s = slabs[si]
                S = G_s * C_s
                et = pool.tile([P, smax], mybir.dt.int8, tag="e")
                nc.sync.dma_start(out=et[:, :S], in_=ev[:, off : off + S])
                xt = pool.tile([P, smax], mybir.dt.float16, tag="x")
                nc.scalar.activation(
                    xt[:, :S],
                    et[:, :S],
                    mybir.ActivationFunctionType.Exp,
                    scale=float(q),
                )
                return xt

            def normalize(si, xt):
                """Stage 2: fold-adds, grouped reduce, recip, multiplies,
                output DMA. Emitted one slab behind load_exp so the ACT
                queue always has the next slab's exp ahead of this slab's
                per-chunk multiplies (keeps the DVE fed)."""
                (off, G_s, C_s), n_act = slabs[si], n_acts[si]
                S = G_s * C_s
                st = pool.tile([P, G_MAX], mybir.dt.float32, tag="s")
                x3 = xt[:, :S].rearrange("p (g c) -> p g c", g=G_s)
                if C_s % 8 == 0 and C_s >= 32:
                    h, hq, he = C_s // 2, C_s // 4, C_s // 8
                    yt = pool.tile([P, smax // 2], mybir.dt.float16, tag="y")
                    y3 = yt[:, : G_s * h].rearrange("p (g c) -> p g c", g=G_s)
                    nc.vector.tensor_add(out=y3, in0=x3[:, :, :h], in1=x3[:, :, h:])
                    zt = pool.tile([P, smax // 4], mybir.dt.float16, tag="z")
                    z3 = zt[:, : G_s * hq].rearrange("p (g c) -> p g c", g=G_s)
                    nc.vector.tensor_add(out=z3, in0=y3[:, :, :hq], in1=y3[:, :, hq:])
                    wt = pool.tile([P, smax // 8], mybir.dt.float16, tag="w")
                    w3 = wt[:, : G_s * he].rearrange("p (g c) -> p g c", g=G_s)
                    nc.vector.tensor_add(out=w3, in0=z3[:, :, :he], in1=z3[:, :, he:])
                    red_in = w3
                elif C_s % 4 == 0 and C_s >= 8:
                    h, hq = C_s // 2, C_s // 4
                    yt = pool.tile([P, smax // 2], mybir.dt.float16, tag="y")
                    y3 = yt[:, : G_s * h].rearrange("p (g c) -> p g c", g=G_s)
                    nc.vector.tensor_add(out=y3, in0=x3[:, :, :h], in1=x3[:, :, h:])
                    zt = pool.tile([P, smax // 4], mybir.dt.float16, tag="z")
                    z3 = zt[:, : G_s * hq].rearrange("p (g c) -> p g c", g=G_s)
                    nc.vector.tensor_add(out=z3, in0=y3[:, :, :hq], in1=y3[:, :, hq:])
                    red_in = z3
                else:
                    red_in = x3
                nc.vector.tensor_reduce(
                    out=st[:, :G_s],
                    in_=red_in,
                    axis=mybir.AxisListType.X,
                    op=mybir.AluOpType.add,
                )
                qt32 = pool.tile([P, G_MAX], mybir.dt.float32, tag="q32")
                nc.vector.reciprocal(out=qt32[:, :G_s], in_=st[:, :G_s])
                # duplicated-pair fp16 reciprocals: the bmul broadcast AP
                # gets a packed (stride-1, count-2) last dim, which keeps
                # the DVE in 2x mode (a plain stride-0 operand drops to 1x)
                qt2 = pool.tile([P, 2 * G_MAX], mybir.dt.float16, tag="q2")
                with nc.allow_low_precision(reason="1/s fits fp16; gate 2e-2"):
                    nc.vector.tensor_copy(
                        out=qt2[:, : 2 * G_s].rearrange("p (g i) -> p g i", g=G_s),
                        in_=qt32[:, :G_s]
                        .rearrange("p g -> p g ()")
                        .to_broadcast([P, G_s, 2]),
                    )
                at = pool.tile([P, smax], mybir.dt.float16, tag="a")
                for g in range(n_act):
                    o = slice(g * C_s, (g + 1) * C_s)
                    nc.scalar.mul(at[:, o], xt[:, o], qt32[:, g : g + 1])
                if n_act < G_s:
                    Gr = G_s - n_act
                    o = slice(n_act * C_s, S)
                    nc.vector.tensor_mul(
                        out=at[:, o].rearrange(
                            "p (g c2 i) -> p g c2 i", g=Gr, i=2
                        ),
                        in0=xt[:, o].rearrange(
                            "p (g c2 i) -> p g c2 i", g=Gr, i=2
                        ),
                        in1=qt2[:, 2 * n_act : 2 * G_s]
                        .rearrange("p (g i) -> p g () i", g=Gr)
                        .to_broadcast([P, Gr, C_s // 2, 2]),
                    )
                nc.sync.dma_start(out=av[:, off : off + S], in_=at[:, :S])

            prev = None
            for si in proc:
                xt = load_exp(si)
                if prev is not None:
                    normalize(*prev)
                prev = (si, xt)
            normalize(*prev)
    nc.compile()
    return nc


def _prepare(e, tgt, num_nodes):
    """Host-side pack: (per-core int8 arrays, scale, scatter metadata)."""
    E = e.shape[0]
    N = num_nodes
    deg = np.bincount(tgt, minlength=N).astype(np.int64)
    order, deg_sorted, slabs, W, chunk_off = _plan(deg, N)

    q = float(np.abs(e).max()) / 127.0
    e8 = np.clip(np.rint(e * (1.0 / q)), -127, 127).astype(np.int8)

    rankpos = np.empty(N, dtype=np.int64)
    rankpos[order] = np.arange(N, dtype=np.int64)
    r = rankpos[tgt]  # [E] degree-rank of each edge's target
    sidx = np.argsort(r, kind="stable")  # edges grouped by rank
    rs = r[sidx]
    starts = np.concatenate(([0], np.cumsum(deg_sorted[:-1])))
    j = np.arange(E, dtype=np.int64) - starts[rs]  # slot within node
    core = rs % NCORES
    pos = rs // NCORES
    gidx = pos // P
    lane = pos % P
    col = chunk_off[gidx] + j
    flat = lane * W + col

    ev = np.full((NCORES, P * W), -128, dtype=np.int8)
    ev[core, flat] = e8[sidx]
    return ev, slabs, W, q, sidx, core, flat


def kernel(e, edge_index, num_nodes):
    from concourse.bass_utils import run_bass_kernel_spmd

    e = np.ascontiguousarray(np.asarray(e, dtype=np.float32))
    tgt = np.asarray(edge_index)[1].astype(np.int64)
    N = int(num_nodes)
    E = e.shape[0]

    ev, slabs, W, q, sidx, core, flat = _prepare(e, tgt, N)

    key = (tuple(slabs), W, round(q, 9))
    if key not in _CACHE:
        _CACHE[key] = _build(slabs, W, q)
    nc = _CACHE[key]

    in_maps = [{"ev": ev[c].reshape(P, W)} for c in range(NCORES)]
    res = run_bass_kernel_spmd(nc, in_maps, core_ids=list(range(NCORES)))

    av = np.stack([res.results[c]["av"].reshape(-1) for c in range(NCORES)])
    alpha = np.empty(E, dtype=np.float32)
    alpha[sidx] = av[core, flat].astype(np.float32)
    return alpha
